# revision 7
# baseline (speedup 1.0000x reference)
"""ConvDeepSet Trainium2 kernel.

Reference op (per batch b):
  D[n, m]   = (x_n - t_m)^2
  K_c[n, m] = exp(-0.5 * D / scale_c^2)          (scale_c = exp(sigma_c))
  dens[m]   = sum_n K_0[n, m]
  conv[m]   = sum_n y_n * K_1[n, m]
  out[m, :] = dens * W[:, 0] + (conv / (dens + 1e-8)) * W[:, 1] + b

Device strategy (data-parallel: 2 batches per NeuronCore, 8 cores):
  m is tiled into 32 chunks of 128 partitions; n (=512) is the free dim.
  Per chunk:
    - PE matmul (K=3) computes a*(t-x)^2 directly into PSUM:
        lhsT rows over m: [a*t^2, 1, t];  rhs rows over n: [1, a*x^2, -2*a*x]
    - ScalarE activation Exp(-D) writes K to SBUF and row-sums into dens
      (fused accum_out).
    - VectorE tensor_tensor_reduce computes K*y and row-sums into conv.
  Then per batch: normalized = conv * recip(dens + eps); a PE transpose turns
  the [128, 32*3] (dens, norm, ones) tile into [96, 128] lhsT rows, and 32
  tiny PE matmuls against [W0; W1; b] produce the [128, 64] output tiles.
"""

import numpy as np

import concourse.bass as bass
import concourse.bacc as bacc
import concourse.tile as tile
import concourse.mybir as mybir
from concourse.bass_utils import run_bass_kernel_spmd
from concourse.masks import make_identity

B, N_IN, N_OUT = 16, 512, 4096
OUT_CH = 64
N_CORES = 8
BPC = B // N_CORES  # batches per core
P = 128
NCHUNK = N_OUT // P  # 32
EPS = 1e-8
F32 = mybir.dt.float32

_CACHE: dict = {}


def _build(shared_scale: bool):
    nc = bacc.Bacc("TRN2", target_bir_lowering=False, debug=False)

    lhs_a = nc.dram_tensor("lhs_a", [BPC, 3, N_OUT], F32, kind="ExternalInput").ap()
    rhs_a = nc.dram_tensor("rhs_a", [BPC, 3, N_IN], F32, kind="ExternalInput").ap()
    if not shared_scale:
        lhs_b = nc.dram_tensor("lhs_b", [BPC, 3, N_OUT], F32, kind="ExternalInput").ap()
        rhs_b = nc.dram_tensor("rhs_b", [BPC, 3, N_IN], F32, kind="ExternalInput").ap()
    y_row = nc.dram_tensor("y_row", [BPC, N_IN], F32, kind="ExternalInput").ap()
    wb_d = nc.dram_tensor("wb", [3, OUT_CH], F32, kind="ExternalInput").ap()
    out_d = nc.dram_tensor("out", [BPC, N_OUT, OUT_CH], F32, kind="ExternalOutput").ap()

    with tile.TileContext(nc) as tc:
        with (
            tc.tile_pool(name="singles", bufs=1) as singles,
            tc.tile_pool(name="perbatch", bufs=2) as perbatch,
            tc.tile_pool(name="kbuf", bufs=4) as kbuf,
            tc.tile_pool(name="scr", bufs=3) as scr,
            tc.tile_pool(name="outbuf", bufs=4) as outbuf,
            tc.tile_pool(name="dps", bufs=4, space="PSUM") as dps,
            tc.tile_pool(name="fps", bufs=1, space="PSUM") as fps,
            tc.tile_pool(name="ops", bufs=3, space="PSUM") as ops,
        ):
            ident = singles.tile([P, P], F32)
            make_identity(nc, ident)
            wb_sb = singles.tile([3, OUT_CH], F32)
            nc.sync.dma_start(out=wb_sb, in_=wb_d)
            eps_sb = singles.tile([P, 1], F32)
            nc.vector.memset(eps_sb, EPS)

            for bb in range(BPC):
                lhsa_sb = perbatch.tile([3, N_OUT], F32, tag="lhsa")
                nc.sync.dma_start(out=lhsa_sb, in_=lhs_a[bb])
                rhsa_sb = perbatch.tile([3, N_IN], F32, tag="rhsa")
                nc.sync.dma_start(out=rhsa_sb, in_=rhs_a[bb])
                if not shared_scale:
                    lhsb_sb = perbatch.tile([3, N_OUT], F32, tag="lhsb")
                    nc.sync.dma_start(out=lhsb_sb, in_=lhs_b[bb])
                    rhsb_sb = perbatch.tile([3, N_IN], F32, tag="rhsb")
                    nc.sync.dma_start(out=rhsb_sb, in_=rhs_b[bb])

                yb_sb = perbatch.tile([P, N_IN], F32, tag="ybcast")
                ya = y_row[bb : bb + 1, :]
                y_bcast = bass.AP(
                    tensor=ya.tensor, offset=ya.offset, ap=[[0, P], ya.ap[-1]]
                )
                nc.gpsimd.dma_start(out=yb_sb, in_=y_bcast)

                # c-major columns: [dens_0..dens_31 | norm_0..norm_31 | ones x32]
                stacked = perbatch.tile([P, 3 * NCHUNK], F32, tag="stacked")
                nc.gpsimd.memset(stacked, 1.0)
                conv_all = perbatch.tile([P, NCHUNK], F32, tag="convall")

                for j in range(NCHUNK):
                    dpsum = dps.tile([P, N_IN], F32, tag="dpsum")
                    nc.tensor.matmul(
                        dpsum,
                        lhsa_sb[:, j * P : (j + 1) * P],
                        rhsa_sb,
                        start=True,
                        stop=True,
                    )
                    k_sb = kbuf.tile([P, N_IN], F32, tag="k")
                    nc.scalar.activation(
                        out=k_sb,
                        in_=dpsum,
                        func=mybir.ActivationFunctionType.Exp,
                        scale=-1.0,
                        accum_out=stacked[:, j : j + 1],
                    )
                    if not shared_scale:
                        dpsum2 = dps.tile([P, N_IN], F32, tag="dpsum2")
                        nc.tensor.matmul(
                            dpsum2,
                            lhsb_sb[:, j * P : (j + 1) * P],
                            rhsb_sb,
                            start=True,
                            stop=True,
                        )
                        k_sb = kbuf.tile([P, N_IN], F32, tag="k2")
                        nc.scalar.activation(
                            out=k_sb,
                            in_=dpsum2,
                            func=mybir.ActivationFunctionType.Exp,
                            scale=-1.0,
                        )
                    scratch = scr.tile([P, N_IN], F32, tag="scratch")
                    nc.vector.scalar_tensor_tensor(
                        out=scratch,
                        in0=k_sb,
                        scalar=1.0,
                        in1=yb_sb,
                        op0=mybir.AluOpType.mult,
                        op1=mybir.AluOpType.mult,
                        accum_out=conv_all[:, j : j + 1],
                    )

                # normalized = conv / (dens + eps)
                denseps = perbatch.tile([P, NCHUNK], F32, tag="denseps")
                nc.scalar.activation(
                    out=denseps,
                    in_=stacked[:, 0:NCHUNK],
                    func=mybir.ActivationFunctionType.Identity,
                    bias=eps_sb,
                )
                rall = perbatch.tile([P, NCHUNK], F32, tag="rall")
                nc.vector.reciprocal(out=rall, in_=denseps)
                nc.vector.tensor_mul(
                    stacked[:, NCHUNK : 2 * NCHUNK], conv_all, rall
                )

                # transpose [128, 96] -> [96, 128]: row c*32+j holds channel c
                # (dens/norm/ones) for chunk j over its 128 m values
                fpsum = fps.tile([3 * NCHUNK, P], F32, tag="fpsum")
                nc.tensor.transpose(fpsum, stacked, ident)
                fT = perbatch.tile([3 * NCHUNK, P], F32, tag="fT")
                nc.scalar.copy(fT, fpsum)
                # repack [96, 128] -> [3, 4096]: row c = channel c over all m
                # (same element order, fewer partitions) so each chunk's lhsT
                # slice below starts at base partition 0
                fTg = perbatch.tile([3, N_OUT], F32, tag="fTg")
                nc.sync.dma_start(out=fTg, in_=fT)

                for j in range(NCHUNK):
                    opsum = ops.tile([P, OUT_CH], F32, tag="opsum")
                    nc.tensor.matmul(
                        opsum,
                        fTg[:, j * P : (j + 1) * P],
                        wb_sb,
                        start=True,
                        stop=True,
                    )
                    osb = outbuf.tile([P, OUT_CH], F32, tag="osb")
                    if j % 2 == 0:
                        nc.scalar.copy(osb, opsum)
                    else:
                        nc.vector.tensor_copy(osb, opsum)
                    nc.sync.dma_start(out=out_d[bb, j * P : (j + 1) * P, :], in_=osb)

    nc.compile()
    return nc


def _prep_in_maps(x, y, t, sigma, W, b):
    x = np.ascontiguousarray(x[..., 0], dtype=np.float32)  # (B, N_IN)
    y = np.ascontiguousarray(y[..., 0], dtype=np.float32)  # (B, N_IN)
    t = np.ascontiguousarray(t[..., 0], dtype=np.float32)  # (B, N_OUT)
    scales = np.exp(sigma.astype(np.float32))
    a0 = np.float32(0.5) / (scales[0] * scales[0])
    a1 = np.float32(0.5) / (scales[1] * scales[1])
    shared = bool(np.float32(a0) == np.float32(a1))

    wb = np.concatenate([W.T.astype(np.float32), b[None, :].astype(np.float32)], axis=0)
    wb = np.ascontiguousarray(wb, dtype=np.float32)  # (3, OUT_CH): [W[:,0], W[:,1], b]

    def lhs_rhs(a):
        lhs = np.stack([a * t * t, np.ones_like(t), t], axis=1)  # (B, 3, N_OUT)
        rhs = np.stack([np.ones_like(x), a * x * x, -2.0 * a * x], axis=1)
        return (
            np.ascontiguousarray(lhs, dtype=np.float32),
            np.ascontiguousarray(rhs, dtype=np.float32),
        )

    lhs_a, rhs_a = lhs_rhs(a0)
    if not shared:
        lhs_b, rhs_b = lhs_rhs(a1)

    in_maps = []
    for c in range(N_CORES):
        sl = slice(c * BPC, (c + 1) * BPC)
        m = {
            "lhs_a": lhs_a[sl],
            "rhs_a": rhs_a[sl],
            "y_row": y[sl],
            "wb": wb,
        }
        if not shared:
            m["lhs_b"] = lhs_b[sl]
            m["rhs_b"] = rhs_b[sl]
        in_maps.append(m)
    return in_maps, shared


def kernel(x, y, t, sigma, W, b, _trace=False):
    in_maps, shared = _prep_in_maps(x, y, t, sigma, W, b)
    if shared not in _CACHE:
        _CACHE[shared] = _build(shared)
    nc = _CACHE[shared]
    res = run_bass_kernel_spmd(
        nc, in_maps, core_ids=list(range(N_CORES)), trace=_trace
    )
    out = np.concatenate([r["out"] for r in res.results], axis=0)
    kernel.last_exec_time_ns = res.exec_time_ns
    kernel.last_results = res
    return np.ascontiguousarray(out.reshape(B, N_OUT, OUT_CH), dtype=np.float32)


# revision 11
# speedup vs baseline: 1.8562x; 1.8562x over previous
"""ConvDeepSet Trainium2 kernel.

Reference op (per batch b):
  D[n, m]   = (x_n - t_m)^2
  K_c[n, m] = exp(-0.5 * D / scale_c^2)          (scale_c = exp(sigma_c))
  dens[m]   = sum_n K_0[n, m]
  conv[m]   = sum_n y_n * K_1[n, m]
  out[m, :] = dens * W[:, 0] + (conv / (dens + 1e-8)) * W[:, 1] + b

Device strategy (data-parallel: 2 batches per NeuronCore, 8 cores):
  m is tiled into 32 chunks of 128 partitions; n (=512) is the free dim.
  Per chunk:
    - PE matmul computes a*(t-x)^2 directly into PSUM. fp32 matmuls run at
      1/4 rate on trn2, so the three terms a*t^2 + a*x^2 + (-2a*t)*x are
      expanded into 12 bf16 rows (3-way hi/mid/lo splits; products of bf16
      pairs are exact in fp32 and the stream cost is K-independent), giving
      bf16 speed at ~1e-5 absolute accuracy in the exponent.
    - ScalarE activation Exp(-D) writes K to SBUF and row-sums into dens
      (fused accum_out).
    - VectorE scalar_tensor_tensor computes K*y and row-sums into conv.
  Then per batch: normalized = conv * recip(dens + eps); dens/norm are
  split into bf16 (hi, lo) pairs, one PE transpose + repack DMAs build the
  [8, 4096] lhsT rows, and 32 small bf16 matmuls against the split
  [W0; W1; b] rows produce [128, 64] output tiles (grouped 4 per PSUM bank
  for one copy + one DMA per group).
"""

import numpy as np
import ml_dtypes

import concourse.bass as bass
import concourse.bacc as bacc
import concourse.tile as tile
import concourse.mybir as mybir
from concourse.bass_utils import run_bass_kernel_spmd
from concourse.masks import make_identity

B, N_IN, N_OUT = 16, 512, 4096
OUT_CH = 64
N_CORES = 8
BPC = B // N_CORES  # batches per core
P = 128
NCHUNK = N_OUT // P  # 32
GROUP = 4  # output chunks per PSUM bank / copy / DMA
EPS = 1e-8
F32 = mybir.dt.float32
BF16 = mybir.dt.bfloat16
BF = ml_dtypes.bfloat16

_CACHE: dict = {}


def _build(shared_scale: bool):
    nc = bacc.Bacc("TRN2", target_bir_lowering=False, debug=False)

    lhs_a = nc.dram_tensor("lhs_a", [BPC, 12, N_OUT], BF16, kind="ExternalInput").ap()
    rhs_a = nc.dram_tensor("rhs_a", [BPC, 12, N_IN], BF16, kind="ExternalInput").ap()
    if not shared_scale:
        lhs_b = nc.dram_tensor(
            "lhs_b", [BPC, 12, N_OUT], BF16, kind="ExternalInput"
        ).ap()
        rhs_b = nc.dram_tensor(
            "rhs_b", [BPC, 12, N_IN], BF16, kind="ExternalInput"
        ).ap()
    y_row = nc.dram_tensor("y_row", [BPC, N_IN], F32, kind="ExternalInput").ap()
    wb_d = nc.dram_tensor("wb8", [8, OUT_CH], BF16, kind="ExternalInput").ap()
    out_d = nc.dram_tensor("out", [BPC, N_OUT, OUT_CH], F32, kind="ExternalOutput").ap()

    with tile.TileContext(nc) as tc:
        with (
            tc.tile_pool(name="singles", bufs=1) as singles,
            tc.tile_pool(name="perbatch", bufs=2) as perbatch,
            tc.tile_pool(name="kbuf", bufs=4) as kbuf,
            tc.tile_pool(name="scr", bufs=3) as scr,
            tc.tile_pool(name="outbuf", bufs=4) as outbuf,
            tc.tile_pool(name="dps", bufs=4, space="PSUM") as dps,
            tc.tile_pool(name="fps", bufs=1, space="PSUM") as fps,
            tc.tile_pool(name="ops", bufs=3, space="PSUM") as ops,
        ):
            ident_bf = singles.tile([P, P], BF16)
            make_identity(nc, ident_bf)
            wb_sb = singles.tile([8, OUT_CH], BF16)
            nc.sync.dma_start(out=wb_sb, in_=wb_d)
            eps_sb = singles.tile([P, 1], F32)
            nc.vector.memset(eps_sb, EPS)

            for bb in range(BPC):
                lhsa_sb = perbatch.tile([12, N_OUT], BF16, tag="lhsa")
                nc.sync.dma_start(out=lhsa_sb, in_=lhs_a[bb])
                rhsa_sb = perbatch.tile([12, N_IN], BF16, tag="rhsa")
                nc.sync.dma_start(out=rhsa_sb, in_=rhs_a[bb])
                if not shared_scale:
                    lhsb_sb = perbatch.tile([12, N_OUT], BF16, tag="lhsb")
                    nc.sync.dma_start(out=lhsb_sb, in_=lhs_b[bb])
                    rhsb_sb = perbatch.tile([12, N_IN], BF16, tag="rhsb")
                    nc.sync.dma_start(out=rhsb_sb, in_=rhs_b[bb])

                yb_sb = perbatch.tile([P, N_IN], F32, tag="ybcast")
                ya = y_row[bb : bb + 1, :]
                y_bcast = bass.AP(
                    tensor=ya.tensor, offset=ya.offset, ap=[[0, P], ya.ap[-1]]
                )
                nc.gpsimd.dma_start(out=yb_sb, in_=y_bcast)

                # f32 per-chunk stats, c-major: [dens 0:32 | norm 32:64]
                stacked = perbatch.tile([P, 2 * NCHUNK], F32, tag="stacked")
                conv_all = perbatch.tile([P, NCHUNK], F32, tag="convall")

                for j in range(NCHUNK):
                    dpsum = dps.tile([P, N_IN], F32, tag="dpsum")
                    nc.tensor.matmul(
                        dpsum,
                        lhsa_sb[:, j * P : (j + 1) * P],
                        rhsa_sb,
                        start=True,
                        stop=True,
                    )
                    k_sb = kbuf.tile([P, N_IN], F32, tag="k")
                    nc.scalar.activation(
                        out=k_sb,
                        in_=dpsum,
                        func=mybir.ActivationFunctionType.Exp,
                        scale=-1.0,
                        accum_out=stacked[:, j : j + 1],
                    )
                    if not shared_scale:
                        dpsum2 = dps.tile([P, N_IN], F32, tag="dpsum2")
                        nc.tensor.matmul(
                            dpsum2,
                            lhsb_sb[:, j * P : (j + 1) * P],
                            rhsb_sb,
                            start=True,
                            stop=True,
                        )
                        k_sb = kbuf.tile([P, N_IN], F32, tag="k2")
                        nc.scalar.activation(
                            out=k_sb,
                            in_=dpsum2,
                            func=mybir.ActivationFunctionType.Exp,
                            scale=-1.0,
                        )
                    scratch = scr.tile([P, N_IN], F32, tag="scratch")
                    nc.vector.scalar_tensor_tensor(
                        out=scratch,
                        in0=k_sb,
                        scalar=1.0,
                        in1=yb_sb,
                        op0=mybir.AluOpType.mult,
                        op1=mybir.AluOpType.mult,
                        accum_out=conv_all[:, j : j + 1],
                    )

                # normalized = conv / (dens + eps)
                denseps = perbatch.tile([P, NCHUNK], F32, tag="denseps")
                nc.scalar.activation(
                    out=denseps,
                    in_=stacked[:, 0:NCHUNK],
                    func=mybir.ActivationFunctionType.Identity,
                    bias=eps_sb,
                )
                rall = perbatch.tile([P, NCHUNK], F32, tag="rall")
                nc.vector.reciprocal(out=rall, in_=denseps)
                nc.vector.tensor_mul(
                    stacked[:, NCHUNK : 2 * NCHUNK], conv_all, rall
                )

                # bf16 hi/lo splits, c-major: [dh | dl | nh | nl]
                sbf = perbatch.tile([P, 4 * NCHUNK], BF16, tag="sbf")
                nc.scalar.copy(sbf[:, 0:NCHUNK], stacked[:, 0:NCHUNK])
                nc.vector.tensor_sub(
                    sbf[:, NCHUNK : 2 * NCHUNK],
                    stacked[:, 0:NCHUNK],
                    sbf[:, 0:NCHUNK],
                )
                nc.scalar.copy(
                    sbf[:, 2 * NCHUNK : 3 * NCHUNK], stacked[:, NCHUNK : 2 * NCHUNK]
                )
                nc.vector.tensor_sub(
                    sbf[:, 3 * NCHUNK : 4 * NCHUNK],
                    stacked[:, NCHUNK : 2 * NCHUNK],
                    sbf[:, 2 * NCHUNK : 3 * NCHUNK],
                )

                # transpose [128, 128] -> [128, 128]: row c*32+j = channel c of
                # chunk j over its 128 m values (c in dh/dl/nh/nl)
                fpsum = fps.tile([4 * NCHUNK, P], BF16, tag="fpsum")
                nc.tensor.transpose(fpsum, sbf, ident_bf)
                fT4 = perbatch.tile([4 * NCHUNK, P], BF16, tag="fT4")
                nc.scalar.copy(fT4, fpsum)

                # repack to [8, 4096] lhsT rows: [1, 1, dh, dh, dl, nh, nh, nl]
                # paired against wb8 rows [bh, bl, W0h, W0l, W0h, W1h, W1l, W1h]
                fTg = perbatch.tile([8, N_OUT], BF16, tag="fTg")
                nc.gpsimd.memset(fTg[0:2, :], 1.0)
                for r, c in ((2, 0), (3, 0), (4, 1), (5, 2), (6, 2), (7, 3)):
                    nc.sync.dma_start(
                        out=fTg[r : r + 1, :],
                        in_=fT4[c * NCHUNK : (c + 1) * NCHUNK, :],
                    )

                for j0 in range(0, NCHUNK, GROUP):
                    opsum = ops.tile([P, GROUP * OUT_CH], F32, tag="opsum")
                    for q in range(GROUP):
                        nc.tensor.matmul(
                            opsum[:, q * OUT_CH : (q + 1) * OUT_CH],
                            fTg[:, (j0 + q) * P : (j0 + q + 1) * P],
                            wb_sb,
                            start=True,
                            stop=True,
                        )
                    osb = outbuf.tile([P, GROUP * OUT_CH], F32, tag="osb")
                    if (j0 // GROUP) % 2 == 0:
                        nc.scalar.copy(osb, opsum)
                    else:
                        nc.vector.tensor_copy(osb, opsum)
                    sub = out_d[bb, j0 * P : (j0 + GROUP) * P, :]
                    dst = bass.AP(
                        tensor=sub.tensor,
                        offset=sub.offset,
                        ap=[[OUT_CH, P], [P * OUT_CH, GROUP], [1, OUT_CH]],
                    )
                    nc.sync.dma_start(out=dst, in_=osb)

    nc.compile()
    return nc


def _split3(v):
    """3-way bf16 hi/mid/lo split of a float64 array."""
    vh = v.astype(BF)
    r1 = v - vh.astype(np.float64)
    vm = r1.astype(BF)
    r2 = r1 - vm.astype(np.float64)
    vl = r2.astype(BF)
    return vh, vm, vl


def _d_rows(a, pts_t, pts_x):
    """12 bf16 lhs rows (over t) and rhs rows (over x) whose pairwise products
    sum to a*(t-x)^2 with ~1e-5 absolute accuracy."""
    t = pts_t.astype(np.float64)
    x = pts_x.astype(np.float64)
    t2h, t2m, t2l = _split3(a * t * t)
    x2h, x2m, x2l = _split3(a * x * x)
    th, tm, tl = _split3(t)
    uh, um, ul = _split3(-2.0 * a * x)
    ones_t = np.ones_like(t, dtype=BF)
    ones_x = np.ones_like(x, dtype=BF)
    lhs = np.stack(
        [t2h, t2m, t2l, ones_t, ones_t, ones_t, th, th, tm, th, tm, tl], axis=-2
    )
    rhs = np.stack(
        [ones_x, ones_x, ones_x, x2h, x2m, x2l, uh, um, uh, ul, um, uh], axis=-2
    )
    return np.ascontiguousarray(lhs), np.ascontiguousarray(rhs)


def _prep_in_maps(x, y, t, sigma, W, b):
    x = np.ascontiguousarray(x[..., 0], dtype=np.float32)  # (B, N_IN)
    y = np.ascontiguousarray(y[..., 0], dtype=np.float32)  # (B, N_IN)
    t = np.ascontiguousarray(t[..., 0], dtype=np.float32)  # (B, N_OUT)
    scales = np.exp(sigma.astype(np.float32))
    a0 = np.float32(0.5) / (scales[0] * scales[0])
    a1 = np.float32(0.5) / (scales[1] * scales[1])
    shared = bool(np.float32(a0) == np.float32(a1))

    # wb8 rows pair with lhsT rows [1, 1, dh, dh, dl, nh, nh, nl]
    w64 = W.astype(np.float64)
    b64 = b.astype(np.float64)
    w0h = w64[:, 0].astype(BF)
    w0l = (w64[:, 0] - w0h.astype(np.float64)).astype(BF)
    w1h = w64[:, 1].astype(BF)
    w1l = (w64[:, 1] - w1h.astype(np.float64)).astype(BF)
    bh = b64.astype(BF)
    bl = (b64 - bh.astype(np.float64)).astype(BF)
    wb8 = np.ascontiguousarray(np.stack([bh, bl, w0h, w0l, w0h, w1h, w1l, w1h]))

    lhs_a, rhs_a = _d_rows(float(a0), t, x)  # (B, 12, N_OUT), (B, 12, N_IN)
    if not shared:
        lhs_b, rhs_b = _d_rows(float(a1), t, x)

    in_maps = []
    for c in range(N_CORES):
        sl = slice(c * BPC, (c + 1) * BPC)
        m = {
            "lhs_a": lhs_a[sl],
            "rhs_a": rhs_a[sl],
            "y_row": y[sl],
            "wb8": wb8,
        }
        if not shared:
            m["lhs_b"] = lhs_b[sl]
            m["rhs_b"] = rhs_b[sl]
        in_maps.append(m)
    return in_maps, shared


def kernel(x, y, t, sigma, W, b, _trace=False):
    in_maps, shared = _prep_in_maps(x, y, t, sigma, W, b)
    if shared not in _CACHE:
        _CACHE[shared] = _build(shared)
    nc = _CACHE[shared]
    res = run_bass_kernel_spmd(
        nc, in_maps, core_ids=list(range(N_CORES)), trace=_trace
    )
    out = np.concatenate([r["out"] for r in res.results], axis=0)
    kernel.last_exec_time_ns = res.exec_time_ns
    kernel.last_results = res
    return np.ascontiguousarray(out.reshape(B, N_OUT, OUT_CH), dtype=np.float32)


# revision 15
# speedup vs baseline: 2.1597x; 1.1635x over previous
"""ConvDeepSet Trainium2 kernel.

Reference op (per batch b):
  D[n, m]   = (x_n - t_m)^2
  K_c[n, m] = exp(-0.5 * D / scale_c^2)          (scale_c = exp(sigma_c))
  dens[m]   = sum_n K_0[n, m]
  conv[m]   = sum_n y_n * K_1[n, m]
  out[m, :] = dens * W[:, 0] + (conv / (dens + 1e-8)) * W[:, 1] + b

Fast path (shared scale, the compiled-for case) uses the Gaussian
convolution identity to factor the kernel through a P=32 grid of RBF
features with O(1e-6) relative aliasing error:

  exp(-a(x-t)^2) = c0 * sum_p phi_p(x) phi_p(t),
  phi_p(u) = exp(-2a(u-g_p)^2),  g_p a uniform grid, c0 = h*sqrt(4a/pi)

so the N_IN-point reduction becomes a 32-feature contraction:

  agg_c[m] = sum_p A[c,p] phi_p(t_m),   A[c,p] = c0 * sum_n Y[n,c] phi_p(x_n)

Device pipeline per batch (data-parallel: 2 batches/core, 8 cores):
  - D1[n,p] = 2a(x_n-g_p)^2 and D2[p,m] = 2a(g_p-t_m)^2 via 12-row bf16
    split-precision matmuls (3-way hi/mid/lo splits; bf16 products are
    exact in fp32; stream cost is K-independent, and fp32 matmuls would
    run at 1/4 rate).
  - Phi_x = exp(-D1 + ln c0) on ScalarE (f32), A accumulated by a tiny
    fp32 matmul against [1|y]; A transposed to [32, 2] via two scatter
    DMAs and split into fp16 (Ah, Al).
  - Phi_t = exp(-D2) on ScalarE (f32 scratch), cast to fp16 phh (GpSimd)
    with fp16 residual phl (VectorE).
  - agg[m, 0:2] per 128-chunk of m = three tiny fp16 matmuls accumulating
    in PSUM: phh'Ah + phl'Ah + phh'Al (fp16 pair arithmetic ~ 2^-22).
  - Finale: normalized = conv * recip(dens+eps); dens/norm split to bf16
    (hi, lo); one PE transpose + repack DMAs build [8, 4096] lhsT rows
    [1, 1, dh, dh, dl, nh, nh, nl] against wb8 rows
    [bh, bl, W0h, W0l, W0h, W1h, W1l, W1h]; 32 small bf16 matmuls produce
    [128, 64] output tiles (grouped 4/PSUM bank: one copy + one DMA each).
"""

import numpy as np
import ml_dtypes

import concourse.bass as bass
import concourse.bacc as bacc
import concourse.tile as tile
import concourse.mybir as mybir
from concourse.bass_utils import run_bass_kernel_spmd
from concourse.masks import make_identity

B, N_IN, N_OUT = 16, 512, 4096
OUT_CH = 64
N_CORES = 8
BPC = B // N_CORES  # batches per core
P = 128
NCHUNK = N_OUT // P  # 32
NXCH = N_IN // P  # 4
MT = 512  # m-tile width for Phi_t generation
NMT = N_OUT // MT  # 8
GRID = 32  # RBF grid points
GROUP = 4  # output chunks per PSUM bank / copy / DMA
EPS = 1e-8
F32 = mybir.dt.float32
BF16 = mybir.dt.bfloat16
FP16 = mybir.dt.float16
F16 = np.float16
BF = ml_dtypes.bfloat16

_CACHE: dict = {}


def _finale(nc, pools, stacked64, wb_sb, ident_bf, eps_sb, out_d, bb):
    """dens/conv [128, 64] (cols 2j, 2j+1) -> normalized, bf16 splits,
    transpose, repack, 32 final matmuls, grouped copies + DMAs out."""
    perbatch, fps, ops, outbuf = pools
    st = stacked64.rearrange("p (j c) -> p j c", c=2)
    dens_cols = st[:, :, 0]
    conv_cols = st[:, :, 1]

    denseps = perbatch.tile([P, NCHUNK], F32, tag="denseps")
    nc.scalar.activation(
        out=denseps,
        in_=dens_cols,
        func=mybir.ActivationFunctionType.Identity,
        bias=eps_sb,
    )
    rall = perbatch.tile([P, NCHUNK], F32, tag="rall")
    nc.vector.reciprocal(out=rall, in_=denseps)
    norm32 = perbatch.tile([P, NCHUNK], F32, tag="norm32")
    nc.vector.tensor_mul(norm32, conv_cols, rall)

    # bf16 hi/lo splits, c-major: [dh | dl | nh | nl]
    sbf = perbatch.tile([P, 4 * NCHUNK], BF16, tag="sbf")
    nc.scalar.copy(sbf[:, 0:NCHUNK], dens_cols)
    nc.vector.tensor_sub(sbf[:, NCHUNK : 2 * NCHUNK], dens_cols, sbf[:, 0:NCHUNK])
    nc.scalar.copy(sbf[:, 2 * NCHUNK : 3 * NCHUNK], norm32)
    nc.vector.tensor_sub(
        sbf[:, 3 * NCHUNK : 4 * NCHUNK], norm32, sbf[:, 2 * NCHUNK : 3 * NCHUNK]
    )

    fpsum = fps.tile([4 * NCHUNK, P], BF16, tag="fpsum")
    nc.tensor.transpose(fpsum, sbf, ident_bf)
    fT4 = perbatch.tile([4 * NCHUNK, P], BF16, tag="fT4")
    nc.scalar.copy(fT4, fpsum)

    # [8, 4096] lhsT rows [1, 1, dh, dh, dl, nh, nh, nl]
    fTg = perbatch.tile([8, N_OUT], BF16, tag="fTg")
    nc.gpsimd.memset(fTg[0:2, :], 1.0)
    for r, c in ((2, 0), (3, 0), (4, 1), (5, 2), (6, 2), (7, 3)):
        nc.sync.dma_start(
            out=fTg[r : r + 1, :], in_=fT4[c * NCHUNK : (c + 1) * NCHUNK, :]
        )

    for j0 in range(0, NCHUNK, GROUP):
        opsum = ops.tile([P, GROUP * OUT_CH], F32, tag="opsum")
        for q in range(GROUP):
            nc.tensor.matmul(
                opsum[:, q * OUT_CH : (q + 1) * OUT_CH],
                fTg[:, (j0 + q) * P : (j0 + q + 1) * P],
                wb_sb,
                start=True,
                stop=True,
            )
        osb = outbuf.tile([P, GROUP * OUT_CH], F32, tag="osb")
        if (j0 // GROUP) % 2 == 0:
            nc.scalar.copy(osb, opsum)
        else:
            nc.vector.tensor_copy(osb, opsum)
        sub = out_d[bb, j0 * P : (j0 + GROUP) * P, :]
        dst = bass.AP(
            tensor=sub.tensor,
            offset=sub.offset,
            ap=[[OUT_CH, P], [P * OUT_CH, GROUP], [1, OUT_CH]],
        )
        nc.sync.dma_start(out=dst, in_=osb)


def _build_rbf(ln_c0: float):
    nc = bacc.Bacc("TRN2", target_bir_lowering=False, debug=False)

    d1_lhs = nc.dram_tensor("d1_lhs", [BPC, 12, N_IN], BF16, kind="ExternalInput").ap()
    d1_rhs = nc.dram_tensor("d1_rhs", [12, GRID], BF16, kind="ExternalInput").ap()
    d2_lhs = nc.dram_tensor("d2_lhs", [12, GRID], BF16, kind="ExternalInput").ap()
    d2_rhs = nc.dram_tensor("d2_rhs", [BPC, 12, N_OUT], BF16, kind="ExternalInput").ap()
    y2_d = nc.dram_tensor("y2", [BPC, P, NXCH, 2], F32, kind="ExternalInput").ap()
    wb_d = nc.dram_tensor("wb8", [8, OUT_CH], BF16, kind="ExternalInput").ap()
    out_d = nc.dram_tensor("out", [BPC, N_OUT, OUT_CH], F32, kind="ExternalOutput").ap()

    with tile.TileContext(nc) as tc:
        with (
            tc.tile_pool(name="singles", bufs=1) as singles,
            tc.tile_pool(name="perbatch", bufs=2) as perbatch,
            tc.tile_pool(name="phi", bufs=3) as phi,
            tc.tile_pool(name="outbuf", bufs=4) as outbuf,
            tc.tile_pool(name="d1psp", bufs=1, space="PSUM") as d1psp,
            tc.tile_pool(name="apsp", bufs=1, space="PSUM") as apsp,
            tc.tile_pool(name="d2psp", bufs=2, space="PSUM") as d2psp,
            tc.tile_pool(name="aggps", bufs=2, space="PSUM") as aggps,
            tc.tile_pool(name="fps", bufs=1, space="PSUM") as fps,
            tc.tile_pool(name="ops", bufs=1, space="PSUM") as ops,
        ):
            ident_bf = singles.tile([P, P], BF16)
            make_identity(nc, ident_bf)
            wb_sb = singles.tile([8, OUT_CH], BF16)
            nc.sync.dma_start(out=wb_sb, in_=wb_d)
            eps_sb = singles.tile([P, 1], F32)
            nc.vector.memset(eps_sb, EPS)
            lnc0_sb = singles.tile([P, 1], F32)
            nc.vector.memset(lnc0_sb, ln_c0)
            d1r_sb = singles.tile([12, GRID], BF16)
            nc.sync.dma_start(out=d1r_sb, in_=d1_rhs)
            d2l_sb = singles.tile([12, GRID], BF16)
            nc.sync.dma_start(out=d2l_sb, in_=d2_lhs)

            for bb in range(BPC):
                d1l_sb = perbatch.tile([12, N_IN], BF16, tag="d1l")
                nc.sync.dma_start(out=d1l_sb, in_=d1_lhs[bb])
                d2r_sb = perbatch.tile([12, N_OUT], BF16, tag="d2r")
                nc.sync.dma_start(out=d2r_sb, in_=d2_rhs[bb])
                y2_sb = perbatch.tile([P, NXCH, 2], F32, tag="y2")
                nc.sync.dma_start(out=y2_sb, in_=y2_d[bb])

                # ---- x side: A[2, 32] = c0 * [1|y]^T Phi_x ----
                aps_t = apsp.tile([2, GRID], F32, tag="aps")
                for jn in range(NXCH):
                    d1ps = d1psp.tile([P, GRID], F32, tag="d1ps")
                    nc.tensor.matmul(
                        d1ps,
                        d1l_sb[:, jn * P : (jn + 1) * P],
                        d1r_sb,
                        start=True,
                        stop=True,
                    )
                    phx = phi.tile([P, GRID], F32, tag="phx")
                    nc.scalar.activation(
                        out=phx,
                        in_=d1ps,
                        func=mybir.ActivationFunctionType.Exp,
                        scale=-1.0,
                        bias=lnc0_sb,
                    )
                    nc.tensor.matmul(
                        aps_t,
                        y2_sb[:, jn, :],
                        phx,
                        start=(jn == 0),
                        stop=(jn == NXCH - 1),
                    )
                a_sb = perbatch.tile([2, GRID], F32, tag="a_sb")
                nc.scalar.copy(a_sb, aps_t)
                a32 = perbatch.tile([GRID, 2], F32, tag="a32")
                for c in range(2):
                    nc.sync.dma_start(out=a32[:, c : c + 1], in_=a_sb[c : c + 1, :])
                ah = perbatch.tile([GRID, 2], FP16, tag="ah")
                nc.scalar.copy(ah, a32)
                al = perbatch.tile([GRID, 2], FP16, tag="al")
                nc.vector.tensor_sub(al, a32, ah)

                # ---- t side: Phi_t fp16 pair [32, 4096] ----
                phh = perbatch.tile([GRID, N_OUT], FP16, tag="phh")
                phl = perbatch.tile([GRID, N_OUT], FP16, tag="phl")
                for mt in range(NMT):
                    d2ps = d2psp.tile([GRID, MT], F32, tag="d2ps")
                    nc.tensor.matmul(
                        d2ps,
                        d2l_sb,
                        d2r_sb[:, mt * MT : (mt + 1) * MT],
                        start=True,
                        stop=True,
                    )
                    phf = phi.tile([GRID, MT], F32, tag="phf")
                    nc.scalar.activation(
                        out=phf,
                        in_=d2ps,
                        func=mybir.ActivationFunctionType.Exp,
                        scale=-1.0,
                    )
                    sl = slice(mt * MT, (mt + 1) * MT)
                    nc.gpsimd.tensor_copy(phh[:, sl], phf)
                    nc.vector.tensor_sub(phl[:, sl], phf, phh[:, sl])

                # ---- agg[m, 0:2] per chunk: phh'Ah + phl'Ah + phh'Al ----
                agg = aggps.tile([P, 2 * NCHUNK], F32, tag="agg")
                for j in range(NCHUNK):
                    sl = slice(j * P, (j + 1) * P)
                    o2 = agg[:, 2 * j : 2 * j + 2]
                    nc.tensor.matmul(o2, phh[:, sl], ah, start=True, stop=False)
                    nc.tensor.matmul(o2, phl[:, sl], ah, start=False, stop=False)
                    nc.tensor.matmul(o2, phh[:, sl], al, start=False, stop=True)
                stacked64 = perbatch.tile([P, 2 * NCHUNK], F32, tag="stacked64")
                nc.scalar.copy(stacked64, agg)

                _finale(
                    nc,
                    (perbatch, fps, ops, outbuf),
                    stacked64,
                    wb_sb,
                    ident_bf,
                    eps_sb,
                    out_d,
                    bb,
                )

    nc.compile()
    return nc


def _build_bruteforce():
    """Fallback for distinct per-channel scales: direct exp over all pairs
    (12-row bf16 split D-matmuls per channel, exp+accum on ScalarE, conv on
    VectorE)."""
    nc = bacc.Bacc("TRN2", target_bir_lowering=False, debug=False)

    lhs_a = nc.dram_tensor("lhs_a", [BPC, 12, N_OUT], BF16, kind="ExternalInput").ap()
    rhs_a = nc.dram_tensor("rhs_a", [BPC, 12, N_IN], BF16, kind="ExternalInput").ap()
    lhs_b = nc.dram_tensor("lhs_b", [BPC, 12, N_OUT], BF16, kind="ExternalInput").ap()
    rhs_b = nc.dram_tensor("rhs_b", [BPC, 12, N_IN], BF16, kind="ExternalInput").ap()
    y_row = nc.dram_tensor("y_row", [BPC, N_IN], F32, kind="ExternalInput").ap()
    wb_d = nc.dram_tensor("wb8", [8, OUT_CH], BF16, kind="ExternalInput").ap()
    out_d = nc.dram_tensor("out", [BPC, N_OUT, OUT_CH], F32, kind="ExternalOutput").ap()

    with tile.TileContext(nc) as tc:
        with (
            tc.tile_pool(name="singles", bufs=1) as singles,
            tc.tile_pool(name="perbatch", bufs=2) as perbatch,
            tc.tile_pool(name="kbuf", bufs=4) as kbuf,
            tc.tile_pool(name="scr", bufs=3) as scr,
            tc.tile_pool(name="outbuf", bufs=4) as outbuf,
            tc.tile_pool(name="dps", bufs=4, space="PSUM") as dps,
            tc.tile_pool(name="fps", bufs=1, space="PSUM") as fps,
            tc.tile_pool(name="ops", bufs=3, space="PSUM") as ops,
        ):
            ident_bf = singles.tile([P, P], BF16)
            make_identity(nc, ident_bf)
            wb_sb = singles.tile([8, OUT_CH], BF16)
            nc.sync.dma_start(out=wb_sb, in_=wb_d)
            eps_sb = singles.tile([P, 1], F32)
            nc.vector.memset(eps_sb, EPS)

            for bb in range(BPC):
                lhsa_sb = perbatch.tile([12, N_OUT], BF16, tag="lhsa")
                nc.sync.dma_start(out=lhsa_sb, in_=lhs_a[bb])
                rhsa_sb = perbatch.tile([12, N_IN], BF16, tag="rhsa")
                nc.sync.dma_start(out=rhsa_sb, in_=rhs_a[bb])
                lhsb_sb = perbatch.tile([12, N_OUT], BF16, tag="lhsb")
                nc.sync.dma_start(out=lhsb_sb, in_=lhs_b[bb])
                rhsb_sb = perbatch.tile([12, N_IN], BF16, tag="rhsb")
                nc.sync.dma_start(out=rhsb_sb, in_=rhs_b[bb])

                yb_sb = perbatch.tile([P, N_IN], F32, tag="ybcast")
                ya = y_row[bb : bb + 1, :]
                y_bcast = bass.AP(
                    tensor=ya.tensor, offset=ya.offset, ap=[[0, P], ya.ap[-1]]
                )
                nc.gpsimd.dma_start(out=yb_sb, in_=y_bcast)

                stacked64 = perbatch.tile([P, 2 * NCHUNK], F32, tag="stacked64")
                for j in range(NCHUNK):
                    dpsum = dps.tile([P, N_IN], F32, tag="dpsum")
                    nc.tensor.matmul(
                        dpsum,
                        lhsa_sb[:, j * P : (j + 1) * P],
                        rhsa_sb,
                        start=True,
                        stop=True,
                    )
                    k_sb = kbuf.tile([P, N_IN], F32, tag="k")
                    nc.scalar.activation(
                        out=k_sb,
                        in_=dpsum,
                        func=mybir.ActivationFunctionType.Exp,
                        scale=-1.0,
                        accum_out=stacked64[:, 2 * j : 2 * j + 1],
                    )
                    dpsum2 = dps.tile([P, N_IN], F32, tag="dpsum2")
                    nc.tensor.matmul(
                        dpsum2,
                        lhsb_sb[:, j * P : (j + 1) * P],
                        rhsb_sb,
                        start=True,
                        stop=True,
                    )
                    k2_sb = kbuf.tile([P, N_IN], F32, tag="k2")
                    nc.scalar.activation(
                        out=k2_sb,
                        in_=dpsum2,
                        func=mybir.ActivationFunctionType.Exp,
                        scale=-1.0,
                    )
                    scratch = scr.tile([P, N_IN], F32, tag="scratch")
                    nc.vector.scalar_tensor_tensor(
                        out=scratch,
                        in0=k2_sb,
                        scalar=1.0,
                        in1=yb_sb,
                        op0=mybir.AluOpType.mult,
                        op1=mybir.AluOpType.mult,
                        accum_out=stacked64[:, 2 * j + 1 : 2 * j + 2],
                    )

                _finale(
                    nc,
                    (perbatch, fps, ops, outbuf),
                    stacked64,
                    wb_sb,
                    ident_bf,
                    eps_sb,
                    out_d,
                    bb,
                )

    nc.compile()
    return nc


def _split3(v):
    """3-way bf16 hi/mid/lo split of a float64 array."""
    vh = v.astype(BF)
    r1 = v - vh.astype(np.float64)
    vm = r1.astype(BF)
    r2 = r1 - vm.astype(np.float64)
    vl = r2.astype(BF)
    return vh, vm, vl


def _d_rows(a, pts_t, pts_x):
    """12 bf16 lhs rows (over pts_t) and rhs rows (over pts_x) whose pairwise
    products sum to a*(t-x)^2 with ~1e-5 absolute accuracy."""
    t = np.asarray(pts_t, dtype=np.float64)
    x = np.asarray(pts_x, dtype=np.float64)
    t2h, t2m, t2l = _split3(a * t * t)
    x2h, x2m, x2l = _split3(a * x * x)
    th, tm, tl = _split3(t)
    uh, um, ul = _split3(-2.0 * a * x)
    ones_t = np.ones_like(t, dtype=BF)
    ones_x = np.ones_like(x, dtype=BF)
    lhs = np.stack(
        [t2h, t2m, t2l, ones_t, ones_t, ones_t, th, th, tm, th, tm, tl], axis=-2
    )
    rhs = np.stack(
        [ones_x, ones_x, ones_x, x2h, x2m, x2l, uh, um, uh, ul, um, uh], axis=-2
    )
    return np.ascontiguousarray(lhs), np.ascontiguousarray(rhs)


def _wb8(W, b):
    w64 = W.astype(np.float64)
    b64 = b.astype(np.float64)
    w0h = w64[:, 0].astype(BF)
    w0l = (w64[:, 0] - w0h.astype(np.float64)).astype(BF)
    w1h = w64[:, 1].astype(BF)
    w1l = (w64[:, 1] - w1h.astype(np.float64)).astype(BF)
    bh = b64.astype(BF)
    bl = (b64 - bh.astype(np.float64)).astype(BF)
    return np.ascontiguousarray(np.stack([bh, bl, w0h, w0l, w0h, w1h, w1l, w1h]))


def _prep_rbf(x, y, t, a0, W, b):
    beta = 2.0 * a0
    s = 1.0 / (2.0 * np.sqrt(a0))
    margin = s * 5.68
    g = np.linspace(-margin, 1.0 + margin, GRID)
    h = g[1] - g[0]
    c0 = h * np.sqrt(4.0 * a0 / np.pi)
    ln_c0 = float(np.log(c0))

    d1_lhs, d1_rhs = _d_rows(beta, x, g)  # (B, 12, N_IN), (12, GRID)
    d2_lhs, d2_rhs = _d_rows(beta, g, t)  # (12, GRID), (B, 12, N_OUT)
    # y2[b, p, chunk, c]: lhsT chunk slices [128, 2] of [1 | y]
    y2 = np.empty((B, P, NXCH, 2), np.float32)
    y2[..., 0] = 1.0
    y2[..., 1] = y.reshape(B, NXCH, P).transpose(0, 2, 1)
    wb8 = _wb8(W, b)

    in_maps = []
    for c in range(N_CORES):
        sl = slice(c * BPC, (c + 1) * BPC)
        in_maps.append(
            {
                "d1_lhs": d1_lhs[sl],
                "d1_rhs": d1_rhs,
                "d2_lhs": d2_lhs,
                "d2_rhs": d2_rhs[sl],
                "y2": np.ascontiguousarray(y2[sl]),
                "wb8": wb8,
            }
        )
    return in_maps, ln_c0


def _prep_bruteforce(x, y, t, a0, a1, W, b):
    lhs_a, rhs_a = _d_rows(float(a0), t, x)
    lhs_b, rhs_b = _d_rows(float(a1), t, x)
    wb8 = _wb8(W, b)
    in_maps = []
    for c in range(N_CORES):
        sl = slice(c * BPC, (c + 1) * BPC)
        in_maps.append(
            {
                "lhs_a": lhs_a[sl],
                "rhs_a": rhs_a[sl],
                "lhs_b": lhs_b[sl],
                "rhs_b": rhs_b[sl],
                "y_row": y[sl],
                "wb8": wb8,
            }
        )
    return in_maps


def kernel(x, y, t, sigma, W, b, _trace=False):
    x = np.ascontiguousarray(x[..., 0], dtype=np.float32)  # (B, N_IN)
    y = np.ascontiguousarray(y[..., 0], dtype=np.float32)  # (B, N_IN)
    t = np.ascontiguousarray(t[..., 0], dtype=np.float32)  # (B, N_OUT)
    scales = np.exp(sigma.astype(np.float32))
    a0 = float(np.float32(0.5) / (scales[0] * scales[0]))
    a1 = float(np.float32(0.5) / (scales[1] * scales[1]))
    shared = a0 == a1

    if shared:
        in_maps, ln_c0 = _prep_rbf(x, y, t, a0, W, b)
        key = ("rbf", ln_c0)
        if key not in _CACHE:
            _CACHE[key] = _build_rbf(ln_c0)
    else:
        in_maps = _prep_bruteforce(x, y, t, a0, a1, W, b)
        key = "bf"
        if key not in _CACHE:
            _CACHE[key] = _build_bruteforce()
    nc = _CACHE[key]
    res = run_bass_kernel_spmd(
        nc, in_maps, core_ids=list(range(N_CORES)), trace=_trace
    )
    out = np.concatenate([r["out"] for r in res.results], axis=0)
    kernel.last_exec_time_ns = res.exec_time_ns
    kernel.last_results = res
    return np.ascontiguousarray(out.reshape(B, N_OUT, OUT_CH), dtype=np.float32)


# revision 24
# speedup vs baseline: 2.7688x; 1.2821x over previous
"""ConvDeepSet Trainium2 kernel.

Reference op (per batch b):
  D[n, m]   = (x_n - t_m)^2
  K_c[n, m] = exp(-0.5 * D / scale_c^2)          (scale_c = exp(sigma_c))
  dens[m]   = sum_n K_0[n, m]
  conv[m]   = sum_n y_n * K_1[n, m]
  out[m, :] = dens * W[:, 0] + (conv / (dens + 1e-8)) * W[:, 1] + b

Fast path (shared scale, the compiled-for case) uses the Gaussian
convolution identity to factor the kernel through a P=32 grid of RBF
features with O(1e-6) relative aliasing error:

  exp(-a(x-t)^2) = c0 * sum_p phi_p(x) phi_p(t),
  phi_p(u) = exp(-2a(u-g_p)^2),  g_p a uniform grid, c0 = h*sqrt(4a/pi)

so the N_IN-point reduction becomes a 32-feature contraction:

  agg_c[m] = sum_p A[c,p] phi_p(t_m),   A[c,p] = c0 * sum_n Y[n,c] phi_p(x_n)

Device pipeline per batch (data-parallel: 2 batches/core, 8 cores):
  - D1[n,p] = 2a(x_n-g_p)^2 and D2[p,m] = 2a(g_p-t_m)^2 via 12-row bf16
    split-precision matmuls (3-way hi/mid/lo splits; bf16 products are
    exact in fp32; stream cost is K-independent, and fp32 matmuls would
    run at 1/4 rate).
  - Phi_x = exp(-D1 + ln c0) on ScalarE (f32), A accumulated by a tiny
    fp32 matmul against [1|y]; A transposed to [32, 2] via two scatter
    DMAs and split into fp16 (Ah, Al).
  - Phi_t = exp(-D2) on ScalarE (f32 scratch), cast to fp16 phh (GpSimd)
    with fp16 residual phl (VectorE).
  - agg[m, 0:2] per 128-chunk of m = three tiny fp16 matmuls accumulating
    in PSUM: phh'Ah + phl'Ah + phh'Al (fp16 pair arithmetic ~ 2^-22).
  - Finale: normalized = conv * recip(dens+eps); dens/norm split to bf16
    (hi, lo); one PE transpose + repack DMAs build [8, 4096] lhsT rows
    [1, 1, dh, dh, dl, nh, nh, nl] against wb8 rows
    [bh, bl, W0h, W0l, W0h, W1h, W1l, W1h]; 32 small bf16 matmuls produce
    [128, 64] output tiles (grouped 4/PSUM bank: one copy + one DMA each).
"""

import numpy as np
import ml_dtypes

import concourse.bass as bass
import concourse.bacc as bacc
import concourse.tile as tile
import concourse.mybir as mybir
from concourse.bass_utils import run_bass_kernel_spmd
from concourse.masks import make_identity

B, N_IN, N_OUT = 16, 512, 4096
OUT_CH = 64
N_CORES = 8
BPC = B // N_CORES  # batches per core
P = 128
NCHUNK = N_OUT // P  # 32
NXCH = N_IN // P  # 4
MT = 512  # m-tile width for Phi_t generation
NMT = N_OUT // MT  # 8
GRID = 32  # RBF grid points
GROUP = 4  # output chunks per PSUM bank / copy / DMA
EPS = 1e-8
F32 = mybir.dt.float32
BF16 = mybir.dt.bfloat16
FP16 = mybir.dt.float16
F16 = np.float16
BF = ml_dtypes.bfloat16

_CACHE: dict = {}


def _finale(nc, pools, stacked64, wb_sb, ident_bf, eps_sb, out_d, bb):
    """dens/conv [128, 64] (cols 2j, 2j+1) -> normalized, bf16 splits,
    transpose, repack, 32 final matmuls, grouped copies + DMAs out."""
    perbatch, fps, ops, outbuf = pools
    st = stacked64.rearrange("p (j c) -> p j c", c=2)
    dens_cols = st[:, :, 0]
    conv_cols = st[:, :, 1]

    denseps = perbatch.tile([P, NCHUNK], F32, tag="denseps")
    nc.scalar.activation(
        out=denseps,
        in_=dens_cols,
        func=mybir.ActivationFunctionType.Identity,
        bias=eps_sb,
    )
    rall = perbatch.tile([P, NCHUNK], F32, tag="rall")
    nc.vector.reciprocal(out=rall, in_=denseps)
    norm32 = perbatch.tile([P, NCHUNK], F32, tag="norm32")
    nc.vector.tensor_mul(norm32, conv_cols, rall)

    # bf16 hi/lo splits, c-major: [dh | dl | nh | nl]
    sbf = perbatch.tile([P, 4 * NCHUNK], BF16, tag="sbf")
    nc.scalar.copy(sbf[:, 0:NCHUNK], dens_cols)
    nc.vector.tensor_sub(sbf[:, NCHUNK : 2 * NCHUNK], dens_cols, sbf[:, 0:NCHUNK])
    nc.scalar.copy(sbf[:, 2 * NCHUNK : 3 * NCHUNK], norm32)
    nc.vector.tensor_sub(
        sbf[:, 3 * NCHUNK : 4 * NCHUNK], norm32, sbf[:, 2 * NCHUNK : 3 * NCHUNK]
    )

    fpsum = fps.tile([4 * NCHUNK, P], BF16, tag="fpsum")
    nc.tensor.transpose(fpsum, sbf, ident_bf)
    fT4 = perbatch.tile([4 * NCHUNK, P], BF16, tag="fT4")
    nc.scalar.copy(fT4, fpsum)

    # [8, 4096] lhsT rows [1, 1, dh, dh, dl, nh, nh, nl]
    fTg = perbatch.tile([8, N_OUT], BF16, tag="fTg")
    nc.gpsimd.memset(fTg[0:2, :], 1.0)
    for r, c in ((2, 0), (3, 0), (4, 1), (5, 2), (6, 2), (7, 3)):
        nc.sync.dma_start(
            out=fTg[r : r + 1, :], in_=fT4[c * NCHUNK : (c + 1) * NCHUNK, :]
        )

    for j0 in range(0, NCHUNK, GROUP):
        opsum = ops.tile([P, GROUP * OUT_CH], F32, tag="opsum")
        for q in range(GROUP):
            nc.tensor.matmul(
                opsum[:, q * OUT_CH : (q + 1) * OUT_CH],
                fTg[:, (j0 + q) * P : (j0 + q + 1) * P],
                wb_sb,
                start=True,
                stop=True,
            )
        osb = outbuf.tile([P, GROUP * OUT_CH], F32, tag="osb")
        if (j0 // GROUP) % 2 == 0:
            nc.scalar.copy(osb, opsum)
        else:
            nc.vector.tensor_copy(osb, opsum)
        sub = out_d[bb, j0 * P : (j0 + GROUP) * P, :]
        dst = bass.AP(
            tensor=sub.tensor,
            offset=sub.offset,
            ap=[[OUT_CH, P], [P * OUT_CH, GROUP], [1, OUT_CH]],
        )
        nc.sync.dma_start(out=dst, in_=osb)


def _build_rbf(ln_c0: float):
    nc = bacc.Bacc("TRN2", target_bir_lowering=False, debug=False)

    d1_lhs = nc.dram_tensor("d1_lhs", [BPC, 12, N_IN], BF16, kind="ExternalInput").ap()
    d1_rhs = nc.dram_tensor("d1_rhs", [12, GRID], BF16, kind="ExternalInput").ap()
    d2_lhs = nc.dram_tensor("d2_lhs", [12, GRID], BF16, kind="ExternalInput").ap()
    d2_rhs = nc.dram_tensor("d2_rhs", [BPC, 12, N_OUT], BF16, kind="ExternalInput").ap()
    y2_d = nc.dram_tensor("y2", [BPC, P, NXCH, 2], F32, kind="ExternalInput").ap()
    wb_d = nc.dram_tensor("wb8", [8, OUT_CH], BF16, kind="ExternalInput").ap()
    out_d = nc.dram_tensor("out", [BPC, N_OUT, OUT_CH], F32, kind="ExternalOutput").ap()

    with tile.TileContext(nc) as tc:
        with (
            tc.tile_pool(name="singles", bufs=1) as singles,
            tc.tile_pool(name="perbatch", bufs=2) as perbatch,
            tc.tile_pool(name="phi", bufs=3) as phi,
            tc.tile_pool(name="outbuf", bufs=4) as outbuf,
            tc.tile_pool(name="d1psp", bufs=1, space="PSUM") as d1psp,
            tc.tile_pool(name="apsp", bufs=1, space="PSUM") as apsp,
            tc.tile_pool(name="d2psp", bufs=1, space="PSUM") as d2psp,
            tc.tile_pool(name="aggps", bufs=1, space="PSUM") as aggps,
            tc.tile_pool(name="fps", bufs=1, space="PSUM") as fps,
            tc.tile_pool(name="ops", bufs=1, space="PSUM") as ops,
        ):
            ident_bf = singles.tile([P, P], BF16)
            make_identity(nc, ident_bf)
            ident_f32 = singles.tile([P, P], F32)
            make_identity(nc, ident_f32)
            wb_sb = singles.tile([8, OUT_CH], BF16)
            nc.sync.dma_start(out=wb_sb, in_=wb_d)
            eps_sb = singles.tile([P, 1], F32)
            nc.vector.memset(eps_sb, EPS)
            lnc0_sb = singles.tile([P, 1], F32)
            nc.vector.memset(lnc0_sb, ln_c0)
            d1r_sb = singles.tile([12, GRID], BF16)
            nc.sync.dma_start(out=d1r_sb, in_=d1_rhs)
            d2l_sb = singles.tile([12, GRID], BF16)
            nc.sync.dma_start(out=d2l_sb, in_=d2_lhs)

            for bb in range(BPC):
                d1l_sb = perbatch.tile([12, N_IN], BF16, tag="d1l")
                nc.sync.dma_start(out=d1l_sb, in_=d1_lhs[bb])
                d2r_sb = perbatch.tile([12, N_OUT], BF16, tag="d2r")
                nc.sync.dma_start(out=d2r_sb, in_=d2_rhs[bb])
                y2_sb = perbatch.tile([P, NXCH, 2], F32, tag="y2")
                nc.sync.dma_start(out=y2_sb, in_=y2_d[bb])

                # ---- x side: A[2, 32] = c0 * [1|y]^T Phi_x ----
                aps_t = apsp.tile([2, GRID], F32, tag="aps")
                for jn in range(NXCH):
                    d1ps = d1psp.tile([P, GRID], F32, tag="d1ps")
                    nc.tensor.matmul(
                        d1ps,
                        d1l_sb[:, jn * P : (jn + 1) * P],
                        d1r_sb,
                        start=True,
                        stop=True,
                    )
                    phx = phi.tile([P, GRID], F32, tag="phx")
                    nc.scalar.activation(
                        out=phx,
                        in_=d1ps,
                        func=mybir.ActivationFunctionType.Exp,
                        scale=-1.0,
                        bias=lnc0_sb,
                    )
                    nc.tensor.matmul(
                        aps_t,
                        y2_sb[:, jn, :],
                        phx,
                        start=(jn == 0),
                        stop=(jn == NXCH - 1),
                    )
                # A transposed to [32, 2] then replicated x4 along partitions:
                # a32[32r + p, c] = A[c, p]
                a_sb = perbatch.tile([2, GRID], F32, tag="a_sb")
                nc.scalar.copy(a_sb, aps_t)
                atp = apsp.tile([GRID, 2], F32, tag="atp")
                nc.tensor.transpose(atp, a_sb, ident_f32[0:2, 0:2])
                a32 = perbatch.tile([P, 2], F32, tag="a32")
                nc.scalar.copy(a32[0:GRID, :], atp)
                for r in range(1, 4):
                    nc.sync.dma_start(
                        out=a32[r * GRID : (r + 1) * GRID, :], in_=a32[0:GRID, :]
                    )
                # quarter-masked A operands: cols 2q hold A only in rows
                # 32q..32q+31, zero elsewhere, so a full K=128 contraction
                # against the packed Phi tiles picks out quarter q
                ahm = perbatch.tile([P, 8], FP16, tag="ahm")
                nc.gpsimd.memset(ahm, 0.0)
                alm = perbatch.tile([P, 8], FP16, tag="alm")
                nc.gpsimd.memset(alm, 0.0)
                for q in range(4):
                    rows = slice(32 * q, 32 * (q + 1))
                    cols = slice(2 * q, 2 * q + 2)
                    nc.scalar.copy(ahm[rows, cols], a32[rows, :])
                    nc.vector.tensor_sub(alm[rows, cols], a32[rows, :], ahm[rows, cols])

                # ---- t side: Phi_t fp16 pair, packed [128, 1024]:
                # row 32q+p, col m' = phi_p(t_{1024q + m'}) ----
                MQ = N_OUT // 4  # 1024
                d2pk = d2psp.tile([P, MQ], F32, tag="d2pk")
                for mt in range(NMT):
                    q, hh = mt // 2, mt % 2
                    nc.tensor.matmul(
                        d2pk[32 * q : 32 * (q + 1), hh * MT : (hh + 1) * MT],
                        d2l_sb,
                        d2r_sb[:, mt * MT : (mt + 1) * MT],
                        start=True,
                        stop=True,
                        tile_position=(0, 32 * q),
                    )
                phf = phi.tile([P, MQ], F32, tag="phf")
                nc.scalar.activation(
                    out=phf,
                    in_=d2pk,
                    func=mybir.ActivationFunctionType.Exp,
                    scale=-1.0,
                )
                phh = perbatch.tile([P, MQ], FP16, tag="phh")
                nc.scalar.copy(phh, phf)
                phl = perbatch.tile([P, MQ], FP16, tag="phl")
                nc.vector.tensor_sub(phl, phf, phh)

                # ---- agg[m, 0:2] per chunk: phh'Ah + phl'Ah + phh'Al ----
                agg = aggps.tile([P, 2 * NCHUNK], F32, tag="agg")
                for j in range(NCHUNK):
                    q, jj = j // 8, j % 8
                    sl = slice(jj * P, (jj + 1) * P)
                    cols = slice(2 * q, 2 * q + 2)
                    o2 = agg[:, 2 * j : 2 * j + 2]
                    nc.tensor.matmul(
                        o2, phh[:, sl], ahm[:, cols], start=True, stop=False
                    )
                    nc.tensor.matmul(
                        o2, phl[:, sl], ahm[:, cols], start=False, stop=False
                    )
                    nc.tensor.matmul(
                        o2, phh[:, sl], alm[:, cols], start=False, stop=True
                    )
                stacked64 = perbatch.tile([P, 2 * NCHUNK], F32, tag="stacked64")
                nc.scalar.copy(stacked64, agg)

                _finale(
                    nc,
                    (perbatch, fps, ops, outbuf),
                    stacked64,
                    wb_sb,
                    ident_bf,
                    eps_sb,
                    out_d,
                    bb,
                )

    nc.compile()
    return nc


def _build_bruteforce():
    """Fallback for distinct per-channel scales: direct exp over all pairs
    (12-row bf16 split D-matmuls per channel, exp+accum on ScalarE, conv on
    VectorE)."""
    nc = bacc.Bacc("TRN2", target_bir_lowering=False, debug=False)

    lhs_a = nc.dram_tensor("lhs_a", [BPC, 12, N_OUT], BF16, kind="ExternalInput").ap()
    rhs_a = nc.dram_tensor("rhs_a", [BPC, 12, N_IN], BF16, kind="ExternalInput").ap()
    lhs_b = nc.dram_tensor("lhs_b", [BPC, 12, N_OUT], BF16, kind="ExternalInput").ap()
    rhs_b = nc.dram_tensor("rhs_b", [BPC, 12, N_IN], BF16, kind="ExternalInput").ap()
    y_row = nc.dram_tensor("y_row", [BPC, N_IN], F32, kind="ExternalInput").ap()
    wb_d = nc.dram_tensor("wb8", [8, OUT_CH], BF16, kind="ExternalInput").ap()
    out_d = nc.dram_tensor("out", [BPC, N_OUT, OUT_CH], F32, kind="ExternalOutput").ap()

    with tile.TileContext(nc) as tc:
        with (
            tc.tile_pool(name="singles", bufs=1) as singles,
            tc.tile_pool(name="perbatch", bufs=2) as perbatch,
            tc.tile_pool(name="kbuf", bufs=4) as kbuf,
            tc.tile_pool(name="scr", bufs=3) as scr,
            tc.tile_pool(name="outbuf", bufs=4) as outbuf,
            tc.tile_pool(name="dps", bufs=4, space="PSUM") as dps,
            tc.tile_pool(name="fps", bufs=1, space="PSUM") as fps,
            tc.tile_pool(name="ops", bufs=3, space="PSUM") as ops,
        ):
            ident_bf = singles.tile([P, P], BF16)
            make_identity(nc, ident_bf)
            wb_sb = singles.tile([8, OUT_CH], BF16)
            nc.sync.dma_start(out=wb_sb, in_=wb_d)
            eps_sb = singles.tile([P, 1], F32)
            nc.vector.memset(eps_sb, EPS)

            for bb in range(BPC):
                lhsa_sb = perbatch.tile([12, N_OUT], BF16, tag="lhsa")
                nc.sync.dma_start(out=lhsa_sb, in_=lhs_a[bb])
                rhsa_sb = perbatch.tile([12, N_IN], BF16, tag="rhsa")
                nc.sync.dma_start(out=rhsa_sb, in_=rhs_a[bb])
                lhsb_sb = perbatch.tile([12, N_OUT], BF16, tag="lhsb")
                nc.sync.dma_start(out=lhsb_sb, in_=lhs_b[bb])
                rhsb_sb = perbatch.tile([12, N_IN], BF16, tag="rhsb")
                nc.sync.dma_start(out=rhsb_sb, in_=rhs_b[bb])

                yb_sb = perbatch.tile([P, N_IN], F32, tag="ybcast")
                ya = y_row[bb : bb + 1, :]
                y_bcast = bass.AP(
                    tensor=ya.tensor, offset=ya.offset, ap=[[0, P], ya.ap[-1]]
                )
                nc.gpsimd.dma_start(out=yb_sb, in_=y_bcast)

                stacked64 = perbatch.tile([P, 2 * NCHUNK], F32, tag="stacked64")
                for j in range(NCHUNK):
                    dpsum = dps.tile([P, N_IN], F32, tag="dpsum")
                    nc.tensor.matmul(
                        dpsum,
                        lhsa_sb[:, j * P : (j + 1) * P],
                        rhsa_sb,
                        start=True,
                        stop=True,
                    )
                    k_sb = kbuf.tile([P, N_IN], F32, tag="k")
                    nc.scalar.activation(
                        out=k_sb,
                        in_=dpsum,
                        func=mybir.ActivationFunctionType.Exp,
                        scale=-1.0,
                        accum_out=stacked64[:, 2 * j : 2 * j + 1],
                    )
                    dpsum2 = dps.tile([P, N_IN], F32, tag="dpsum2")
                    nc.tensor.matmul(
                        dpsum2,
                        lhsb_sb[:, j * P : (j + 1) * P],
                        rhsb_sb,
                        start=True,
                        stop=True,
                    )
                    k2_sb = kbuf.tile([P, N_IN], F32, tag="k2")
                    nc.scalar.activation(
                        out=k2_sb,
                        in_=dpsum2,
                        func=mybir.ActivationFunctionType.Exp,
                        scale=-1.0,
                    )
                    scratch = scr.tile([P, N_IN], F32, tag="scratch")
                    nc.vector.scalar_tensor_tensor(
                        out=scratch,
                        in0=k2_sb,
                        scalar=1.0,
                        in1=yb_sb,
                        op0=mybir.AluOpType.mult,
                        op1=mybir.AluOpType.mult,
                        accum_out=stacked64[:, 2 * j + 1 : 2 * j + 2],
                    )

                _finale(
                    nc,
                    (perbatch, fps, ops, outbuf),
                    stacked64,
                    wb_sb,
                    ident_bf,
                    eps_sb,
                    out_d,
                    bb,
                )

    nc.compile()
    return nc


def _split3(v):
    """3-way bf16 hi/mid/lo split of a float64 array."""
    vh = v.astype(BF)
    r1 = v - vh.astype(np.float64)
    vm = r1.astype(BF)
    r2 = r1 - vm.astype(np.float64)
    vl = r2.astype(BF)
    return vh, vm, vl


def _d_rows(a, pts_t, pts_x):
    """12 bf16 lhs rows (over pts_t) and rhs rows (over pts_x) whose pairwise
    products sum to a*(t-x)^2 with ~1e-5 absolute accuracy."""
    t = np.asarray(pts_t, dtype=np.float64)
    x = np.asarray(pts_x, dtype=np.float64)
    t2h, t2m, t2l = _split3(a * t * t)
    x2h, x2m, x2l = _split3(a * x * x)
    th, tm, tl = _split3(t)
    uh, um, ul = _split3(-2.0 * a * x)
    ones_t = np.ones_like(t, dtype=BF)
    ones_x = np.ones_like(x, dtype=BF)
    lhs = np.stack(
        [t2h, t2m, t2l, ones_t, ones_t, ones_t, th, th, tm, th, tm, tl], axis=-2
    )
    rhs = np.stack(
        [ones_x, ones_x, ones_x, x2h, x2m, x2l, uh, um, uh, ul, um, uh], axis=-2
    )
    return np.ascontiguousarray(lhs), np.ascontiguousarray(rhs)


def _wb8(W, b):
    w64 = W.astype(np.float64)
    b64 = b.astype(np.float64)
    w0h = w64[:, 0].astype(BF)
    w0l = (w64[:, 0] - w0h.astype(np.float64)).astype(BF)
    w1h = w64[:, 1].astype(BF)
    w1l = (w64[:, 1] - w1h.astype(np.float64)).astype(BF)
    bh = b64.astype(BF)
    bl = (b64 - bh.astype(np.float64)).astype(BF)
    return np.ascontiguousarray(np.stack([bh, bl, w0h, w0l, w0h, w1h, w1l, w1h]))


def _prep_rbf(x, y, t, a0, W, b):
    beta = 2.0 * a0
    s = 1.0 / (2.0 * np.sqrt(a0))
    margin = s * 5.68
    g = np.linspace(-margin, 1.0 + margin, GRID)
    h = g[1] - g[0]
    c0 = h * np.sqrt(4.0 * a0 / np.pi)
    ln_c0 = float(np.log(c0))

    d1_lhs, d1_rhs = _d_rows(beta, x, g)  # (B, 12, N_IN), (12, GRID)
    d2_lhs, d2_rhs = _d_rows(beta, g, t)  # (12, GRID), (B, 12, N_OUT)
    # y2[b, p, chunk, c]: lhsT chunk slices [128, 2] of [1 | y]
    y2 = np.empty((B, P, NXCH, 2), np.float32)
    y2[..., 0] = 1.0
    y2[..., 1] = y.reshape(B, NXCH, P).transpose(0, 2, 1)
    wb8 = _wb8(W, b)

    in_maps = []
    for c in range(N_CORES):
        sl = slice(c * BPC, (c + 1) * BPC)
        in_maps.append(
            {
                "d1_lhs": d1_lhs[sl],
                "d1_rhs": d1_rhs,
                "d2_lhs": d2_lhs,
                "d2_rhs": d2_rhs[sl],
                "y2": np.ascontiguousarray(y2[sl]),
                "wb8": wb8,
            }
        )
    return in_maps, ln_c0


def _prep_bruteforce(x, y, t, a0, a1, W, b):
    lhs_a, rhs_a = _d_rows(float(a0), t, x)
    lhs_b, rhs_b = _d_rows(float(a1), t, x)
    wb8 = _wb8(W, b)
    in_maps = []
    for c in range(N_CORES):
        sl = slice(c * BPC, (c + 1) * BPC)
        in_maps.append(
            {
                "lhs_a": lhs_a[sl],
                "rhs_a": rhs_a[sl],
                "lhs_b": lhs_b[sl],
                "rhs_b": rhs_b[sl],
                "y_row": y[sl],
                "wb8": wb8,
            }
        )
    return in_maps


def kernel(x, y, t, sigma, W, b, _trace=False):
    x = np.ascontiguousarray(x[..., 0], dtype=np.float32)  # (B, N_IN)
    y = np.ascontiguousarray(y[..., 0], dtype=np.float32)  # (B, N_IN)
    t = np.ascontiguousarray(t[..., 0], dtype=np.float32)  # (B, N_OUT)
    scales = np.exp(sigma.astype(np.float32))
    a0 = float(np.float32(0.5) / (scales[0] * scales[0]))
    a1 = float(np.float32(0.5) / (scales[1] * scales[1]))
    shared = a0 == a1

    if shared:
        in_maps, ln_c0 = _prep_rbf(x, y, t, a0, W, b)
        key = ("rbf", ln_c0)
        if key not in _CACHE:
            _CACHE[key] = _build_rbf(ln_c0)
    else:
        in_maps = _prep_bruteforce(x, y, t, a0, a1, W, b)
        key = "bf"
        if key not in _CACHE:
            _CACHE[key] = _build_bruteforce()
    nc = _CACHE[key]
    res = run_bass_kernel_spmd(
        nc, in_maps, core_ids=list(range(N_CORES)), trace=_trace
    )
    out = np.concatenate([r["out"] for r in res.results], axis=0)
    kernel.last_exec_time_ns = res.exec_time_ns
    kernel.last_results = res
    return np.ascontiguousarray(out.reshape(B, N_OUT, OUT_CH), dtype=np.float32)


# revision 26
# speedup vs baseline: 2.9545x; 1.0671x over previous
"""ConvDeepSet Trainium2 kernel.

Reference op (per batch b):
  D[n, m]   = (x_n - t_m)^2
  K_c[n, m] = exp(-0.5 * D / scale_c^2)          (scale_c = exp(sigma_c))
  dens[m]   = sum_n K_0[n, m]
  conv[m]   = sum_n y_n * K_1[n, m]
  out[m, :] = dens * W[:, 0] + (conv / (dens + 1e-8)) * W[:, 1] + b

Fast path (shared scale, the compiled-for case) uses the Gaussian
convolution identity to factor the kernel through a P=32 grid of RBF
features with O(1e-6) relative aliasing error:

  exp(-a(x-t)^2) = c0 * sum_p phi_p(x) phi_p(t),
  phi_p(u) = exp(-2a(u-g_p)^2),  g_p a uniform grid, c0 = h*sqrt(4a/pi)

so the N_IN-point reduction becomes a 32-feature contraction:

  agg_c[m] = sum_p A[c,p] phi_p(t_m),   A[c,p] = c0 * sum_n Y[n,c] phi_p(x_n)

Device pipeline per batch (data-parallel: 2 batches/core, 8 cores):
  - D1[n,p] = 2a(x_n-g_p)^2 and D2[p,m] = 2a(g_p-t_m)^2 via 12-row bf16
    split-precision matmuls (3-way hi/mid/lo splits; bf16 products are
    exact in fp32; stream cost is K-independent, and fp32 matmuls would
    run at 1/4 rate).
  - Phi_x = exp(-D1 + ln c0) on ScalarE (f32), A accumulated by a tiny
    fp32 matmul against [1|y]; A transposed to [32, 2] via two scatter
    DMAs and split into fp16 (Ah, Al).
  - Phi_t = exp(-D2) on ScalarE (f32 scratch), cast to fp16 phh (GpSimd)
    with fp16 residual phl (VectorE).
  - agg[m, 0:2] per 128-chunk of m = three tiny fp16 matmuls accumulating
    in PSUM: phh'Ah + phl'Ah + phh'Al (fp16 pair arithmetic ~ 2^-22).
  - Finale: normalized = conv * recip(dens+eps); dens/norm split to bf16
    (hi, lo); one PE transpose + repack DMAs build [6, 4096] lhsT rows
    [dh, dh, dl, nh, nh, nl] against wb6 rows [W0h, W0l, W0h, W1h, W1l, W1h];
    32 small bf16 matmuls produce [128, 64] output tiles (grouped 8/PSUM
    bank: one bias-add copy + one DMA each).
"""

import numpy as np
import ml_dtypes

import concourse.bass as bass
import concourse.bacc as bacc
import concourse.tile as tile
import concourse.mybir as mybir
from concourse.bass_utils import run_bass_kernel_spmd
from concourse.masks import make_identity

B, N_IN, N_OUT = 16, 512, 4096
OUT_CH = 64
N_CORES = 8
BPC = B // N_CORES  # batches per core
P = 128
NCHUNK = N_OUT // P  # 32
NXCH = N_IN // P  # 4
MT = 512  # m-tile width for Phi_t generation
NMT = N_OUT // MT  # 8
GRID = 32  # RBF grid points
GROUP = 8  # output chunks per PSUM bank / copy / DMA
EPS = 1e-8
F32 = mybir.dt.float32
BF16 = mybir.dt.bfloat16
FP16 = mybir.dt.float16
F16 = np.float16
BF = ml_dtypes.bfloat16

_CACHE: dict = {}


def _finale(nc, pools, stacked64, wb_sb, bb8_sb, ident_bf, eps_sb, out_d, bb):
    """dens/conv [128, 64] (cols 2j, 2j+1) -> normalized, bf16 splits,
    transpose, repack, 32 final matmuls, grouped bias-add copies + DMAs."""
    perbatch, fps, ops, outbuf = pools
    st = stacked64.rearrange("p (j c) -> p j c", c=2)
    dens_cols = st[:, :, 0]
    conv_cols = st[:, :, 1]

    denseps = perbatch.tile([P, NCHUNK], F32, tag="denseps")
    nc.scalar.activation(
        out=denseps,
        in_=dens_cols,
        func=mybir.ActivationFunctionType.Identity,
        bias=eps_sb,
    )
    rall = perbatch.tile([P, NCHUNK], F32, tag="rall")
    nc.vector.reciprocal(out=rall, in_=denseps)
    norm32 = perbatch.tile([P, NCHUNK], F32, tag="norm32")
    nc.vector.tensor_mul(norm32, conv_cols, rall)

    # bf16 hi/lo splits, c-major: [dh | dl | nh | nl]
    sbf = perbatch.tile([P, 4 * NCHUNK], BF16, tag="sbf")
    nc.scalar.copy(sbf[:, 0:NCHUNK], dens_cols)
    nc.vector.tensor_sub(sbf[:, NCHUNK : 2 * NCHUNK], dens_cols, sbf[:, 0:NCHUNK])
    nc.scalar.copy(sbf[:, 2 * NCHUNK : 3 * NCHUNK], norm32)
    nc.vector.tensor_sub(
        sbf[:, 3 * NCHUNK : 4 * NCHUNK], norm32, sbf[:, 2 * NCHUNK : 3 * NCHUNK]
    )

    fpsum = fps.tile([4 * NCHUNK, P], BF16, tag="fpsum")
    nc.tensor.transpose(fpsum, sbf, ident_bf)
    fT4 = perbatch.tile([4 * NCHUNK, P], BF16, tag="fT4")
    nc.scalar.copy(fT4, fpsum)

    # [6, 4096] lhsT rows [dh, dh, dl, nh, nh, nl] paired against wb6 rows
    # [W0h, W0l, W0h, W1h, W1l, W1h]; bias is added at the output copy
    fTg = perbatch.tile([6, N_OUT], BF16, tag="fTg")
    nc.sync.dma_start(out=fTg[0:1, :], in_=fT4[0:NCHUNK, :])
    nc.sync.dma_start(out=fTg[1:2, :], in_=fT4[0:NCHUNK, :])
    nc.sync.dma_start(out=fTg[2:4, :], in_=fT4[NCHUNK : 3 * NCHUNK, :])
    nc.sync.dma_start(out=fTg[4:6, :], in_=fT4[2 * NCHUNK : 4 * NCHUNK, :])

    for j0 in range(0, NCHUNK, GROUP):
        opsum = ops.tile([P, GROUP * OUT_CH], F32, tag="opsum")
        for q in range(GROUP):
            nc.tensor.matmul(
                opsum[:, q * OUT_CH : (q + 1) * OUT_CH],
                fTg[:, (j0 + q) * P : (j0 + q + 1) * P],
                wb_sb,
                start=True,
                stop=True,
            )
        osb = outbuf.tile([P, GROUP * OUT_CH], F32, tag="osb")
        nc.vector.tensor_add(osb, opsum, bb8_sb)
        sub = out_d[bb, j0 * P : (j0 + GROUP) * P, :]
        dst = bass.AP(
            tensor=sub.tensor,
            offset=sub.offset,
            ap=[[OUT_CH, P], [P * OUT_CH, GROUP], [1, OUT_CH]],
        )
        nc.sync.dma_start(out=dst, in_=osb)


def _build_rbf(ln_c0: float):
    nc = bacc.Bacc("TRN2", target_bir_lowering=False, debug=False)

    d1_lhs = nc.dram_tensor("d1_lhs", [BPC, 12, N_IN], BF16, kind="ExternalInput").ap()
    d1_rhs = nc.dram_tensor("d1_rhs", [12, GRID], BF16, kind="ExternalInput").ap()
    d2_lhs = nc.dram_tensor("d2_lhs", [12, GRID], BF16, kind="ExternalInput").ap()
    d2_rhs = nc.dram_tensor("d2_rhs", [BPC, 12, N_OUT], BF16, kind="ExternalInput").ap()
    y2_d = nc.dram_tensor("y2", [BPC, P, NXCH, 2], F32, kind="ExternalInput").ap()
    wb_d = nc.dram_tensor("wb6", [6, OUT_CH], BF16, kind="ExternalInput").ap()
    bb_d = nc.dram_tensor("b_bcast", [P, GROUP * OUT_CH], F32, kind="ExternalInput").ap()
    out_d = nc.dram_tensor("out", [BPC, N_OUT, OUT_CH], F32, kind="ExternalOutput").ap()

    with tile.TileContext(nc) as tc:
        with (
            tc.tile_pool(name="singles", bufs=1) as singles,
            tc.tile_pool(name="perbatch", bufs=2) as perbatch,
            tc.tile_pool(name="phi", bufs=3) as phi,
            tc.tile_pool(name="outbuf", bufs=4) as outbuf,
            tc.tile_pool(name="d1psp", bufs=1, space="PSUM") as d1psp,
            tc.tile_pool(name="apsp", bufs=1, space="PSUM") as apsp,
            tc.tile_pool(name="d2psp", bufs=1, space="PSUM") as d2psp,
            tc.tile_pool(name="aggps", bufs=1, space="PSUM") as aggps,
            tc.tile_pool(name="fps", bufs=1, space="PSUM") as fps,
            tc.tile_pool(name="ops", bufs=1, space="PSUM") as ops,
        ):
            ident_bf = singles.tile([P, P], BF16)
            make_identity(nc, ident_bf)
            ident_f32 = singles.tile([P, P], F32)
            make_identity(nc, ident_f32)
            wb_sb = singles.tile([6, OUT_CH], BF16)
            nc.sync.dma_start(out=wb_sb, in_=wb_d)
            bb8_sb = singles.tile([P, GROUP * OUT_CH], F32)
            nc.sync.dma_start(out=bb8_sb, in_=bb_d)
            eps_sb = singles.tile([P, 1], F32)
            nc.vector.memset(eps_sb, EPS)
            lnc0_sb = singles.tile([P, 1], F32)
            nc.vector.memset(lnc0_sb, ln_c0)
            d1r_sb = singles.tile([12, GRID], BF16)
            nc.sync.dma_start(out=d1r_sb, in_=d1_rhs)
            d2l_sb = singles.tile([12, GRID], BF16)
            nc.sync.dma_start(out=d2l_sb, in_=d2_lhs)

            for bb in range(BPC):
                d1l_sb = perbatch.tile([12, N_IN], BF16, tag="d1l")
                nc.sync.dma_start(out=d1l_sb, in_=d1_lhs[bb])
                d2r_sb = perbatch.tile([12, N_OUT], BF16, tag="d2r")
                nc.sync.dma_start(out=d2r_sb, in_=d2_rhs[bb])
                y2_sb = perbatch.tile([P, NXCH, 2], F32, tag="y2")
                nc.sync.dma_start(out=y2_sb, in_=y2_d[bb])

                # ---- x side: A[2, 32] = c0 * [1|y]^T Phi_x ----
                aps_t = apsp.tile([2, GRID], F32, tag="aps")
                for jn in range(NXCH):
                    d1ps = d1psp.tile([P, GRID], F32, tag="d1ps")
                    nc.tensor.matmul(
                        d1ps,
                        d1l_sb[:, jn * P : (jn + 1) * P],
                        d1r_sb,
                        start=True,
                        stop=True,
                    )
                    phx = phi.tile([P, GRID], F32, tag="phx")
                    nc.scalar.activation(
                        out=phx,
                        in_=d1ps,
                        func=mybir.ActivationFunctionType.Exp,
                        scale=-1.0,
                        bias=lnc0_sb,
                    )
                    nc.tensor.matmul(
                        aps_t,
                        y2_sb[:, jn, :],
                        phx,
                        start=(jn == 0),
                        stop=(jn == NXCH - 1),
                    )
                # A transposed to [32, 2] then replicated x4 along partitions:
                # a32[32r + p, c] = A[c, p]
                a_sb = perbatch.tile([2, GRID], F32, tag="a_sb")
                nc.scalar.copy(a_sb, aps_t)
                atp = apsp.tile([GRID, 2], F32, tag="atp")
                nc.tensor.transpose(atp, a_sb, ident_f32[0:2, 0:2])
                a32 = perbatch.tile([P, 2], F32, tag="a32")
                nc.scalar.copy(a32[0:GRID, :], atp)
                for r in range(1, 4):
                    nc.sync.dma_start(
                        out=a32[r * GRID : (r + 1) * GRID, :], in_=a32[0:GRID, :]
                    )
                # quarter-masked A operands: cols 2q hold A only in rows
                # 32q..32q+31, zero elsewhere, so a full K=128 contraction
                # against the packed Phi tiles picks out quarter q
                ahm = perbatch.tile([P, 8], FP16, tag="ahm")
                nc.vector.memset(ahm, 0.0)
                alm = perbatch.tile([P, 8], FP16, tag="alm")
                nc.vector.memset(alm, 0.0)
                for q in range(4):
                    rows = slice(32 * q, 32 * (q + 1))
                    cols = slice(2 * q, 2 * q + 2)
                    nc.scalar.copy(ahm[rows, cols], a32[rows, :])
                    nc.vector.tensor_sub(alm[rows, cols], a32[rows, :], ahm[rows, cols])

                # ---- t side: Phi_t fp16 pair, packed [128, 1024]:
                # row 32q+p, col m' = phi_p(t_{1024q + m'}) ----
                MQ = N_OUT // 4  # 1024
                d2pk = d2psp.tile([P, MQ], F32, tag="d2pk")
                for mt in range(NMT):
                    q, hh = mt // 2, mt % 2
                    nc.tensor.matmul(
                        d2pk[32 * q : 32 * (q + 1), hh * MT : (hh + 1) * MT],
                        d2l_sb,
                        d2r_sb[:, mt * MT : (mt + 1) * MT],
                        start=True,
                        stop=True,
                        tile_position=(0, 32 * q),
                    )
                phf = phi.tile([P, MQ], F32, tag="phf")
                nc.scalar.activation(
                    out=phf,
                    in_=d2pk,
                    func=mybir.ActivationFunctionType.Exp,
                    scale=-1.0,
                )
                phh = perbatch.tile([P, MQ], FP16, tag="phh")
                nc.scalar.copy(phh, phf)
                phl = perbatch.tile([P, MQ], FP16, tag="phl")
                nc.vector.tensor_sub(phl, phf, phh)

                # ---- agg[m, 0:2] per chunk: phh'Ah + phl'Ah + phh'Al ----
                agg = aggps.tile([P, 2 * NCHUNK], F32, tag="agg")
                for j in range(NCHUNK):
                    q, jj = j // 8, j % 8
                    sl = slice(jj * P, (jj + 1) * P)
                    cols = slice(2 * q, 2 * q + 2)
                    o2 = agg[:, 2 * j : 2 * j + 2]
                    nc.tensor.matmul(
                        o2, phh[:, sl], ahm[:, cols], start=True, stop=False
                    )
                    nc.tensor.matmul(
                        o2, phl[:, sl], ahm[:, cols], start=False, stop=False
                    )
                    nc.tensor.matmul(
                        o2, phh[:, sl], alm[:, cols], start=False, stop=True
                    )
                stacked64 = perbatch.tile([P, 2 * NCHUNK], F32, tag="stacked64")
                nc.scalar.copy(stacked64, agg)

                _finale(
                    nc,
                    (perbatch, fps, ops, outbuf),
                    stacked64,
                    wb_sb,
                    bb8_sb,
                    ident_bf,
                    eps_sb,
                    out_d,
                    bb,
                )

    nc.compile()
    return nc


def _build_bruteforce():
    """Fallback for distinct per-channel scales: direct exp over all pairs
    (12-row bf16 split D-matmuls per channel, exp+accum on ScalarE, conv on
    VectorE)."""
    nc = bacc.Bacc("TRN2", target_bir_lowering=False, debug=False)

    lhs_a = nc.dram_tensor("lhs_a", [BPC, 12, N_OUT], BF16, kind="ExternalInput").ap()
    rhs_a = nc.dram_tensor("rhs_a", [BPC, 12, N_IN], BF16, kind="ExternalInput").ap()
    lhs_b = nc.dram_tensor("lhs_b", [BPC, 12, N_OUT], BF16, kind="ExternalInput").ap()
    rhs_b = nc.dram_tensor("rhs_b", [BPC, 12, N_IN], BF16, kind="ExternalInput").ap()
    y_row = nc.dram_tensor("y_row", [BPC, N_IN], F32, kind="ExternalInput").ap()
    wb_d = nc.dram_tensor("wb6", [6, OUT_CH], BF16, kind="ExternalInput").ap()
    bb_d = nc.dram_tensor("b_bcast", [P, GROUP * OUT_CH], F32, kind="ExternalInput").ap()
    out_d = nc.dram_tensor("out", [BPC, N_OUT, OUT_CH], F32, kind="ExternalOutput").ap()

    with tile.TileContext(nc) as tc:
        with (
            tc.tile_pool(name="singles", bufs=1) as singles,
            tc.tile_pool(name="perbatch", bufs=2) as perbatch,
            tc.tile_pool(name="kbuf", bufs=4) as kbuf,
            tc.tile_pool(name="scr", bufs=3) as scr,
            tc.tile_pool(name="outbuf", bufs=4) as outbuf,
            tc.tile_pool(name="dps", bufs=4, space="PSUM") as dps,
            tc.tile_pool(name="fps", bufs=1, space="PSUM") as fps,
            tc.tile_pool(name="ops", bufs=3, space="PSUM") as ops,
        ):
            ident_bf = singles.tile([P, P], BF16)
            make_identity(nc, ident_bf)
            wb_sb = singles.tile([6, OUT_CH], BF16)
            nc.sync.dma_start(out=wb_sb, in_=wb_d)
            bb8_sb = singles.tile([P, GROUP * OUT_CH], F32)
            nc.sync.dma_start(out=bb8_sb, in_=bb_d)
            eps_sb = singles.tile([P, 1], F32)
            nc.vector.memset(eps_sb, EPS)

            for bb in range(BPC):
                lhsa_sb = perbatch.tile([12, N_OUT], BF16, tag="lhsa")
                nc.sync.dma_start(out=lhsa_sb, in_=lhs_a[bb])
                rhsa_sb = perbatch.tile([12, N_IN], BF16, tag="rhsa")
                nc.sync.dma_start(out=rhsa_sb, in_=rhs_a[bb])
                lhsb_sb = perbatch.tile([12, N_OUT], BF16, tag="lhsb")
                nc.sync.dma_start(out=lhsb_sb, in_=lhs_b[bb])
                rhsb_sb = perbatch.tile([12, N_IN], BF16, tag="rhsb")
                nc.sync.dma_start(out=rhsb_sb, in_=rhs_b[bb])

                yb_sb = perbatch.tile([P, N_IN], F32, tag="ybcast")
                ya = y_row[bb : bb + 1, :]
                y_bcast = bass.AP(
                    tensor=ya.tensor, offset=ya.offset, ap=[[0, P], ya.ap[-1]]
                )
                nc.gpsimd.dma_start(out=yb_sb, in_=y_bcast)

                stacked64 = perbatch.tile([P, 2 * NCHUNK], F32, tag="stacked64")
                for j in range(NCHUNK):
                    dpsum = dps.tile([P, N_IN], F32, tag="dpsum")
                    nc.tensor.matmul(
                        dpsum,
                        lhsa_sb[:, j * P : (j + 1) * P],
                        rhsa_sb,
                        start=True,
                        stop=True,
                    )
                    k_sb = kbuf.tile([P, N_IN], F32, tag="k")
                    nc.scalar.activation(
                        out=k_sb,
                        in_=dpsum,
                        func=mybir.ActivationFunctionType.Exp,
                        scale=-1.0,
                        accum_out=stacked64[:, 2 * j : 2 * j + 1],
                    )
                    dpsum2 = dps.tile([P, N_IN], F32, tag="dpsum2")
                    nc.tensor.matmul(
                        dpsum2,
                        lhsb_sb[:, j * P : (j + 1) * P],
                        rhsb_sb,
                        start=True,
                        stop=True,
                    )
                    k2_sb = kbuf.tile([P, N_IN], F32, tag="k2")
                    nc.scalar.activation(
                        out=k2_sb,
                        in_=dpsum2,
                        func=mybir.ActivationFunctionType.Exp,
                        scale=-1.0,
                    )
                    scratch = scr.tile([P, N_IN], F32, tag="scratch")
                    nc.vector.scalar_tensor_tensor(
                        out=scratch,
                        in0=k2_sb,
                        scalar=1.0,
                        in1=yb_sb,
                        op0=mybir.AluOpType.mult,
                        op1=mybir.AluOpType.mult,
                        accum_out=stacked64[:, 2 * j + 1 : 2 * j + 2],
                    )

                _finale(
                    nc,
                    (perbatch, fps, ops, outbuf),
                    stacked64,
                    wb_sb,
                    bb8_sb,
                    ident_bf,
                    eps_sb,
                    out_d,
                    bb,
                )

    nc.compile()
    return nc


def _split3(v):
    """3-way bf16 hi/mid/lo split of a float64 array."""
    vh = v.astype(BF)
    r1 = v - vh.astype(np.float64)
    vm = r1.astype(BF)
    r2 = r1 - vm.astype(np.float64)
    vl = r2.astype(BF)
    return vh, vm, vl


def _d_rows(a, pts_t, pts_x):
    """12 bf16 lhs rows (over pts_t) and rhs rows (over pts_x) whose pairwise
    products sum to a*(t-x)^2 with ~1e-5 absolute accuracy."""
    t = np.asarray(pts_t, dtype=np.float64)
    x = np.asarray(pts_x, dtype=np.float64)
    t2h, t2m, t2l = _split3(a * t * t)
    x2h, x2m, x2l = _split3(a * x * x)
    th, tm, tl = _split3(t)
    uh, um, ul = _split3(-2.0 * a * x)
    ones_t = np.ones_like(t, dtype=BF)
    ones_x = np.ones_like(x, dtype=BF)
    lhs = np.stack(
        [t2h, t2m, t2l, ones_t, ones_t, ones_t, th, th, tm, th, tm, tl], axis=-2
    )
    rhs = np.stack(
        [ones_x, ones_x, ones_x, x2h, x2m, x2l, uh, um, uh, ul, um, uh], axis=-2
    )
    return np.ascontiguousarray(lhs), np.ascontiguousarray(rhs)


def _wb6(W, b):
    w64 = W.astype(np.float64)
    w0h = w64[:, 0].astype(BF)
    w0l = (w64[:, 0] - w0h.astype(np.float64)).astype(BF)
    w1h = w64[:, 1].astype(BF)
    w1l = (w64[:, 1] - w1h.astype(np.float64)).astype(BF)
    wb6 = np.ascontiguousarray(np.stack([w0h, w0l, w0h, w1h, w1l, w1h]))
    b_bcast = np.ascontiguousarray(
        np.tile(b.astype(np.float32)[None, :], (P, GROUP))
    )
    return wb6, b_bcast


def _prep_rbf(x, y, t, a0, W, b):
    beta = 2.0 * a0
    s = 1.0 / (2.0 * np.sqrt(a0))
    margin = s * 5.68
    g = np.linspace(-margin, 1.0 + margin, GRID)
    h = g[1] - g[0]
    c0 = h * np.sqrt(4.0 * a0 / np.pi)
    ln_c0 = float(np.log(c0))

    d1_lhs, d1_rhs = _d_rows(beta, x, g)  # (B, 12, N_IN), (12, GRID)
    d2_lhs, d2_rhs = _d_rows(beta, g, t)  # (12, GRID), (B, 12, N_OUT)
    # y2[b, p, chunk, c]: lhsT chunk slices [128, 2] of [1 | y]
    y2 = np.empty((B, P, NXCH, 2), np.float32)
    y2[..., 0] = 1.0
    y2[..., 1] = y.reshape(B, NXCH, P).transpose(0, 2, 1)
    wb6, b_bcast = _wb6(W, b)

    in_maps = []
    for c in range(N_CORES):
        sl = slice(c * BPC, (c + 1) * BPC)
        in_maps.append(
            {
                "d1_lhs": d1_lhs[sl],
                "d1_rhs": d1_rhs,
                "d2_lhs": d2_lhs,
                "d2_rhs": d2_rhs[sl],
                "y2": np.ascontiguousarray(y2[sl]),
                "wb6": wb6,
                "b_bcast": b_bcast,
            }
        )
    return in_maps, ln_c0


def _prep_bruteforce(x, y, t, a0, a1, W, b):
    lhs_a, rhs_a = _d_rows(float(a0), t, x)
    lhs_b, rhs_b = _d_rows(float(a1), t, x)
    wb6, b_bcast = _wb6(W, b)
    in_maps = []
    for c in range(N_CORES):
        sl = slice(c * BPC, (c + 1) * BPC)
        in_maps.append(
            {
                "lhs_a": lhs_a[sl],
                "rhs_a": rhs_a[sl],
                "lhs_b": lhs_b[sl],
                "rhs_b": rhs_b[sl],
                "y_row": y[sl],
                "wb6": wb6,
                "b_bcast": b_bcast,
            }
        )
    return in_maps


def kernel(x, y, t, sigma, W, b, _trace=False):
    x = np.ascontiguousarray(x[..., 0], dtype=np.float32)  # (B, N_IN)
    y = np.ascontiguousarray(y[..., 0], dtype=np.float32)  # (B, N_IN)
    t = np.ascontiguousarray(t[..., 0], dtype=np.float32)  # (B, N_OUT)
    scales = np.exp(sigma.astype(np.float32))
    a0 = float(np.float32(0.5) / (scales[0] * scales[0]))
    a1 = float(np.float32(0.5) / (scales[1] * scales[1]))
    shared = a0 == a1

    if shared:
        in_maps, ln_c0 = _prep_rbf(x, y, t, a0, W, b)
        key = ("rbf", ln_c0)
        if key not in _CACHE:
            _CACHE[key] = _build_rbf(ln_c0)
    else:
        in_maps = _prep_bruteforce(x, y, t, a0, a1, W, b)
        key = "bf"
        if key not in _CACHE:
            _CACHE[key] = _build_bruteforce()
    nc = _CACHE[key]
    res = run_bass_kernel_spmd(
        nc, in_maps, core_ids=list(range(N_CORES)), trace=_trace
    )
    out = np.concatenate([r["out"] for r in res.results], axis=0)
    kernel.last_exec_time_ns = res.exec_time_ns
    kernel.last_results = res
    return np.ascontiguousarray(out.reshape(B, N_OUT, OUT_CH), dtype=np.float32)
